# revision 1
# baseline (speedup 1.0000x reference)
"""Trainium2 Bass kernel for multi-head self-attention.

Problem: B=4, S=2048, D=1024, H=16 heads (HD=64), fp32 I/O.
  qkv = x @ w_qkv + b_qkv ; attention(softmax(q k^T / 8) v) ; out @ w_out + b_out

Sharding over 8 NeuronCores: core c handles batch b=c//2 and heads
half=c%2 (8 heads each).  Each core computes a partial output
(its heads' contribution to out[b] @ w_out); the host sums the two
partials per batch and adds the constant bias terms.

Matmul operands are fp16 (DT) by default; scores are accumulated in
fp32 PSUM, exp() runs on ScalarE in fp32 with the 1/sqrt(HD) scale
folded in, and softmax is computed unnormalized with the row-sum from a
fused ones-column in the PV matmul, normalized at the [64, S] stage.
"""

import contextlib
import numpy as np

import concourse.bacc as bacc
import concourse.tile as tile
from concourse.tile_rust import add_dep_helper
from concourse import mybir
from concourse.bass_utils import run_bass_kernel_spmd

B, S, D, H, HD = 4, 2048, 1024, 16, 64
NCORES = 8
NH = 8            # heads per core
QF = 512          # q features per core (= NH * HD), same for k and v
PC = 512          # position chunk (psum bank, fp32)
NPC = S // PC     # 4 position chunks
KT = S // 128     # 16 key-position tiles
DC = D // 128     # 8 contraction chunks
FT_QK = (2 * QF) // 128   # 8 feature tiles of qk
FT_AT = QF // 128         # 4 feature tiles of attn output

F32R = mybir.dt.float32r
F32 = mybir.dt.float32
F16 = mybir.dt.float16
DT = F16          # matmul operand dtype (F16 or F32R)
DT_NP = np.float16 if DT == F16 else np.float32

_CACHE = {}


def _build(repeat=1):
    nc = bacc.Bacc("TRN2", target_bir_lowering=False, debug=False)

    xT = nc.dram_tensor("xT", [D, S], DT, kind="ExternalInput").ap()
    wqk = nc.dram_tensor("wqk", [D, 2 * QF], DT, kind="ExternalInput").ap()
    wv = nc.dram_tensor("wv", [D, QF], DT, kind="ExternalInput").ap()
    bqk = nc.dram_tensor("bqk", [2 * QF, 1], F32, kind="ExternalInput").ap()
    wo = nc.dram_tensor("wo", [QF, D], DT, kind="ExternalInput").ap()
    sel2 = nc.dram_tensor("sel2", [2, 128], F32R, kind="ExternalInput").ap()
    out_d = nc.dram_tensor("out_partial", [S, D], F32, kind="ExternalOutput").ap()

    with tile.TileContext(nc) as tc:
        with contextlib.ExitStack() as ctx:
            with nc.allow_low_precision(reason="f32r/fp16 intermediates are intentional"):
                token = None
                for _ in range(repeat):
                    token = _emit(nc, tc, ctx, xT, wqk, wv, bqk, wo, sel2, out_d,
                                  token=token)
    nc.compile()
    return nc


def _emit(nc, tc, ctx, xT, wqk, wv, bqk, wo, sel2, out_d, token=None):
    with contextlib.ExitStack() as kctx:
        return _emit_inner(nc, tc, kctx, xT, wqk, wv, bqk, wo, sel2, out_d, token)


def _emit_inner(nc, tc, ctx, xT, wqk, wv, bqk, wo, sel2, out_d, token=None):
    # ---- long-lived tensors -------------------------------------------------
    keep = ctx.enter_context(tc.tile_pool(name="keep", bufs=1))
    qkT = keep.tile([128, FT_QK, S], DT, tag="qkT")            # 32 KB/p (fp16)
    v_sb = keep.tile([128, KT, NH, HD + 1], F16, tag="v_sb")   # 16.3 KB/p
    attn = keep.tile([128, FT_AT, S], DT, tag="attn")          # 16 KB/p (fp16)
    bqk_t = keep.tile([128, FT_QK, 1], F32, tag="bqk")
    sel_t = keep.tile([128, 2, 128], F32R, tag="sel")
    wo_t = keep.tile([128, FT_AT, D], DT, tag="wo_t")          # 8 KB/p (fp16)
    xt = keep.tile([128, DC, S], DT, tag="xt")                 # 32 KB/p (fp16)
    wv_t = keep.tile([128, DC, QF], DT, tag="wv_t")            # 8 KB/p
    E_sb0 = keep.tile([128, KT, 2, PC], F16, tag="E_sb0")      # 32 KB/p
    E_sb1 = keep.tile([128, KT, 2, PC], F16, tag="E_sb1")      # 32 KB/p
    E_bufs = (E_sb0, E_sb1)

    wqk_pool = ctx.enter_context(tc.tile_pool(name="wqk_pool", bufs=2))
    rec_pool = ctx.enter_context(tc.tile_pool(name="rec_pool", bufs=1))
    rb_pool = ctx.enter_context(tc.tile_pool(name="rb_pool", bufs=2))
    stg = ctx.enter_context(tc.tile_pool(name="stg", bufs=2))
    ps = ctx.enter_context(tc.tile_pool(name="ps", bufs=4, space="PSUM"))

    def emit_input_dmas():
        first_dmas = []
        first_dmas.append(nc.sync.dma_start(
            out=bqk_t, in_=bqk.rearrange("(ft p) o -> p ft o", p=128)))
        first_dmas.append(nc.sync.dma_start(
            out=sel_t[0:1, :, :],
            in_=sel2.rearrange("a b -> (a b)")[None, :].rearrange(
                "o (a b) -> o a b", a=2)))
        # x^T arrives position-chunk-major so compute can start early
        for pc in range(NPC):
            for dc in range(DC):
                first_dmas.append(nc.sync.dma_start(
                    out=xt[:, dc, pc * PC:(pc + 1) * PC],
                    in_=xT[dc * 128:(dc + 1) * 128, pc * PC:(pc + 1) * PC]))
        for dc in range(DC):
            first_dmas.append(nc.sync.dma_start(
                out=wv_t[:, dc, :], in_=wv[dc * 128:(dc + 1) * 128, :]))
        for fc in range(FT_AT):
            first_dmas.append(nc.sync.dma_start(
                out=wo_t[:, fc, :], in_=wo[fc * 128:(fc + 1) * 128, :]))
        if token is not None:
            for i in first_dmas:
                add_dep_helper(token.ins, i.ins, sync=True,
                               reason="serialize benchmark repeats")

    def b1_dma(ft):
        """fetch one feature tile of the qk weights."""
        wqk_t = wqk_pool.tile([128, DC, 128], DT, tag="wqk_t", name=f"wqk{ft}")
        for dc in range(DC):
            i = nc.sync.dma_start(
                out=wqk_t[:, dc, :],
                in_=wqk[dc * 128:(dc + 1) * 128, ft * 128:(ft + 1) * 128])
            if token is not None:
                add_dep_helper(token.ins, i.ins, sync=True,
                               reason="serialize benchmark repeats")
        return wqk_t

    def b1_mm(wqk_t, ft, pc2):
        """qkT[f, s] = sum_d wqk[d, f] * xT[d, s]  (+ bias), 2 pos chunks."""
        qp = ps.tile([128, 2, PC], F32, tag="ps", name=f"qkps{ft}_{pc2}")
        for dc in range(DC):
            for j in range(2):
                pc = pc2 * 2 + j
                nc.tensor.matmul(
                    qp[:, j, :], wqk_t[:, dc, :],
                    xt[:, dc, pc * PC:(pc + 1) * PC],
                    start=(dc == 0), stop=(dc == DC - 1))
        for j in range(2):
            pc = pc2 * 2 + j
            nc.vector.tensor_scalar_add(
                out=qkT[:, ft, pc * PC:(pc + 1) * PC],
                in0=qp[:, j, :], scalar1=bqk_t[:, ft, :])

    def b1_ft(ft):
        wqk_t = b1_dma(ft)
        for pc2 in range(NPC // 2):
            b1_mm(wqk_t, ft, pc2)

    def b2_block(st2):
        """v[s, f] natural layout (+ ones column), two position tiles."""
        vp = ps.tile([128, 2, PC], F32, tag="ps", name=f"vps{st2}")
        for dc in range(DC):
            for j in range(2):
                st = st2 * 2 + j
                nc.tensor.matmul(
                    vp[:, j, :], xt[:, dc, st * 128:(st + 1) * 128], wv_t[:, dc, :],
                    start=(dc == 0), stop=(dc == DC - 1))
        for j in range(2):
            st = st2 * 2 + j
            nc.vector.tensor_copy(
                out=v_sb[:, st, :, 0:HD],
                in_=vp[:, j, :].rearrange("p (h d) -> p h d", h=NH))

    def emit_sc(E_sb, pp, qc, kt):
        """scores^T matmuls + exp for one kt tile of block (pp, qc)."""
        kft = FT_AT + pp
        qft = pp
        qs = slice(qc * PC, (qc + 1) * PC)
        ks = slice(kt * 128, (kt + 1) * 128)
        sc = ps.tile([128, 2, PC], F32, tag="ps", name=f"sc{pp}_{qc}_{kt}")
        nc.tensor.matmul(
            sc[:, 0, :], qkT[0:64, kft, ks], qkT[0:64, qft, qs],
            start=True, stop=True)
        nc.tensor.matmul(
            sc[:, 1, :], qkT[64:128, kft, ks], qkT[64:128, qft, qs],
            start=True, stop=True)
        nc.scalar.activation(
            out=E_sb[:, kt, :, :], in_=sc,
            func=mybir.ActivationFunctionType.Exp, scale=0.125)

    def emit_pv(E_sb, pv, pp, kt):
        """PV (+ fused row-sum) matmuls for one kt tile of block (pp, qc)."""
        nc.tensor.matmul(
            pv[0:HD + 1, 0, :], v_sb[:, kt, 2 * pp, :], E_sb[:, kt, 0, :],
            start=(kt == 0), stop=(kt == KT - 1))
        nc.tensor.matmul(
            pv[0:HD + 1, 1, :], v_sb[:, kt, 2 * pp + 1, :], E_sb[:, kt, 1, :],
            start=(kt == 0), stop=(kt == KT - 1))

    def emit_norm(pv, pp, qc):
        """normalize: attn[:, pp] = pv[0:64] / rowsum  (both heads)."""
        qs = slice(qc * PC, (qc + 1) * PC)
        rec = rec_pool.tile([128, 2, PC], F32R, tag="rec")
        nc.vector.reciprocal(out=rec[0:1, 0, :], in_=pv[HD:HD + 1, 0, :])
        nc.vector.reciprocal(out=rec[0:1, 1, :], in_=pv[HD:HD + 1, 1, :])
        rb = ps.tile([128, 2, PC], F32, tag="ps", name=f"rb{pp}_{qc}")
        nc.tensor.matmul(rb[:, 0, :], sel_t[0:1, 0, :], rec[0:1, 0, :],
                         start=True, stop=False)
        nc.tensor.matmul(rb[:, 0, :], sel_t[0:1, 1, :], rec[0:1, 1, :],
                         start=False, stop=True)
        rb_sb = rb_pool.tile([128, PC], F32R, tag="rb_sb")
        nc.vector.tensor_copy(out=rb_sb, in_=rb[:, 0, :])
        nc.vector.tensor_mul(
            out=attn[0:64, pp, qs], in0=pv[0:HD, 0, :], in1=rb_sb[0:64, :])
        nc.vector.tensor_mul(
            out=attn[64:128, pp, qs], in0=pv[0:HD, 1, :], in1=rb_sb[64:128, :])

    last_copy = None

    def d_block(qc):
        """output projection for one q-chunk (4 position tiles)."""
        nonlocal last_copy
        for j in range(4):
            st = qc * 4 + j
            ss = slice(st * 128, (st + 1) * 128)
            op = ps.tile([128, 2, PC], F32, tag="ps", name=f"ops{st}")
            for fc in range(FT_AT):
                for n in range(2):
                    nc.tensor.matmul(
                        op[:, n, :], attn[:, fc, ss],
                        wo_t[:, fc, n * PC:(n + 1) * PC],
                        start=(fc == 0), stop=(fc == FT_AT - 1))
            ot = stg.tile([128, D], F32, tag="ot")
            last_copy = nc.vector.tensor_copy(
                out=ot, in_=op.rearrange("p a b -> p (a b)"))
            nc.sync.dma_start(out=out_d[ss, :], in_=ot)

    # ---- emission schedule ---------------------------------------------------
    # Tile directs dependencies by emission order, so every producer must be
    # emitted before its consumer.  Attention blocks form a depth-2 software
    # pipeline at kt granularity: block i's PV matmuls interleave with block
    # i+1's scores+exp, so ScalarE (exp) never starves while TensorE works.
    # Projection feature tiles are emitted as fillers early in each head
    # pair; the output projection for q-chunk qc runs inside the last pair.
    nc.vector.memset(v_sb[:, :, :, HD:HD + 1], 1.0)
    # weight tiles for the first head pair go out before the bulk input,
    # and only the position chunks the first scores need are computed before
    # the first exp, so ScalarE starts as early as possible
    w0 = b1_dma(0)
    w4 = b1_dma(FT_AT + 0)
    emit_input_dmas()
    b1_mm(w0, 0, 0)
    b1_mm(w4, FT_AT + 0, 0)
    # remaining qk weight-tile fetches, in use order
    wts = {0: w0, FT_AT: w4}
    for f in (1, FT_AT + 1, 2, FT_AT + 2, 3, FT_AT + 3):
        wts[f] = b1_dma(f)
    blocks = [(pp, qc) for pp in range(NH // 2) for qc in range(NPC)]
    # spread remaining projection compute, one half-feature-tile per block;
    # a unit assigned to block b is emitted inside b's kt loop, and every
    # consumer of its qkT slice reads it in block b+1 or later
    slots = {(1, 0): 0, (1, 1): 1,
             (FT_AT + 1, 0): 1, (FT_AT + 1, 1): 2,
             (2, 0): 3, (2, 1): 4,
             (FT_AT + 2, 0): 5, (FT_AT + 2, 1): 6,
             (3, 0): 7, (3, 1): 8,
             (FT_AT + 3, 0): 9, (FT_AT + 3, 1): 10}
    fillers = {}
    for (f, pc2), bi in slots.items():
        fillers.setdefault(bi, []).append(
            lambda f=f, pc2=pc2: b1_mm(wts[f], f, pc2))
    # prologue: scores+exp of block 0, with the remaining projection chunks
    # and the v projection interleaved (all must precede block 0's PV)
    for kt in range(KT):
        if kt == 6:
            b1_mm(w4, FT_AT + 0, 1)   # k positions 1024:2048, before kt=8
        if kt == 10:
            b1_mm(w0, 0, 1)           # q positions 1024:2048, before qc=2
        emit_sc(E_bufs[0], 0, 0, kt)
        if kt % 2 == 1:
            b2_block(kt // 2)
    for i, (pp, qc) in enumerate(blocks):
        nxt = blocks[i + 1] if i + 1 < len(blocks) else None
        fl = fillers.get(i, [])
        pv = ps.tile([128, 2, PC], F32, tag="ps", name=f"pv{pp}_{qc}")
        for kt in range(KT):
            emit_pv(E_bufs[i % 2], pv, pp, kt)
            if nxt is not None:
                emit_sc(E_bufs[(i + 1) % 2], nxt[0], nxt[1], kt)
            if kt == 5 and len(fl) > 0:
                fl[0]()
            if kt == 11 and len(fl) > 1:
                fl[1]()
        emit_norm(pv, pp, qc)
        if pp == NH // 2 - 1:
            d_block(qc)
    return last_copy


def _get_nc():
    if "nc" not in _CACHE:
        _CACHE["nc"] = _build()
    return _CACHE["nc"]


def _make_in_maps(x, w_qkv, b_qkv, w_out):
    sel2 = np.zeros((2, 128), dtype=np.float32)
    sel2[0, 0:64] = 1.0
    sel2[1, 64:128] = 1.0
    in_maps = []
    for c in range(NCORES):
        b, half = divmod(c, 2)
        hs = half * QF
        in_maps.append({
            "xT": np.ascontiguousarray(x[b].T).astype(DT_NP),
            "wqk": np.concatenate([w_qkv[:, hs:hs + QF],
                                   w_qkv[:, D + hs:D + hs + QF]],
                                  axis=1).astype(DT_NP),
            "wv": np.ascontiguousarray(w_qkv[:, 2 * D + hs:2 * D + hs + QF]).astype(DT_NP),
            "bqk": np.concatenate([b_qkv[hs:hs + QF],
                                   b_qkv[D + hs:D + hs + QF]])[:, None].astype(np.float32),
            "wo": np.ascontiguousarray(w_out[hs:hs + QF, :]).astype(DT_NP),
            "sel2": sel2,
        })
    return in_maps


def kernel(x, w_qkv, b_qkv, w_out, b_out):
    x = np.asarray(x, dtype=np.float32)
    w_qkv = np.asarray(w_qkv, dtype=np.float32)
    b_qkv = np.asarray(b_qkv, dtype=np.float32)
    w_out = np.asarray(w_out, dtype=np.float32)
    b_out = np.asarray(b_out, dtype=np.float32)

    nc = _get_nc()
    in_maps = _make_in_maps(x, w_qkv, b_qkv, w_out)
    res = run_bass_kernel_spmd(nc, in_maps, list(range(NCORES)))
    _CACHE["last_results"] = res

    # host combine: out[b] = partial_A + partial_B + (b_out + bv @ w_out)
    const = b_out + b_qkv[2 * D:] @ w_out            # [D]
    out = np.empty((B, S, D), dtype=np.float32)
    for b in range(B):
        out[b] = (res.results[2 * b]["out_partial"]
                  + res.results[2 * b + 1]["out_partial"] + const)
    return out



# revision 6
# speedup vs baseline: 1.1264x; 1.1264x over previous
"""Trainium2 Bass kernel for multi-head self-attention (v3).

Problem: B=4, S=2048, D=1024, H=16 heads (HD=64), fp32 I/O.
Sharding: core c handles batch c//2, head-half c%2 (8 heads each); host
sums the two partial outputs per batch and adds constant bias terms.

v3 (cost-model driven, fp16 datapath for accuracy):
- all matmuls fp16 (fp8 weight/score noise does not average away
  relative to attention-output magnitude and blows the 2e-2 budget).
- exp split between ScalarE (exact exp) and VectorE (Schraudolph
  bit-trick: fp16 bits = round(s*184.66 + 15316), ~2% rms on 6/16 kt).
- PV transposed: out[q128, 65] with fused ones-column rowsum
  (65 cycles/instr instead of 512 for the natural layout);
  normalize via per-partition reciprocal+mul into attn^T;
  transpose back via XBAR DMA transpose (no PE, no PSUM).
- fp16 partial-output DMA; host combines in fp32.
"""

import contextlib
import numpy as np

import concourse.bacc as bacc
import concourse.tile as tile
from concourse import mybir
from concourse.bass_utils import run_bass_kernel_spmd

B, S, D, H, HD = 4, 2048, 1024, 16, 64
NCORES = 8
NH = 8              # heads per core
QF = 512            # q features per core
KT = 16             # 128-wide key position tiles
QC = 4              # 512-wide q chunks
HP = 4              # head pairs
F32 = mybir.dt.float32
F16 = mybir.dt.float16
U16 = mybir.dt.uint16
Exp = mybir.ActivationFunctionType.Exp
Identity = mybir.ActivationFunctionType.Identity

LOG2E = 1.4426950408889634
SCH_A = 0.125 * 1024 * LOG2E    # schraudolph fp16-bits slope
SCH_B = 15360.0 - 44.0          # bias 15, centered correction
ACT_KTS = 10                    # kt < ACT_KTS -> exp on ScalarE, else DVE

_CACHE = {}


def _build():
    nc = bacc.Bacc("TRN2", target_bir_lowering=False, debug=False)

    xT = nc.dram_tensor("xT", [D, S], F16, kind="ExternalInput").ap()
    wqk = nc.dram_tensor("wqk", [D, 2 * QF], F16, kind="ExternalInput").ap()
    wv = nc.dram_tensor("wv", [D, QF], F16, kind="ExternalInput").ap()
    wo = nc.dram_tensor("wo", [QF, D], F16, kind="ExternalInput").ap()
    bqk = nc.dram_tensor("bqk", [2 * QF, 1], F32, kind="ExternalInput").ap()
    out_d = nc.dram_tensor("out_partial", [S, D], F16, kind="ExternalOutput").ap()

    with tile.TileContext(nc) as tc:
        with contextlib.ExitStack() as ctx:
            with nc.allow_low_precision(reason="fp16 intermediates"):
                _emit(nc, tc, ctx, xT, wqk, wv, wo, bqk, out_d)
    nc.compile()
    return nc


def _emit(nc, tc, ctx, xT, wqk, wv, wo, bqk, out_d):
    keep = ctx.enter_context(tc.tile_pool(name="keep", bufs=1))
    xt = keep.tile([128, 8, S], F16, tag="xt")              # 32 KB/p
    qkT = keep.tile([128, 8, S], F16, tag="qkT")            # 32 KB/p
    v16 = keep.tile([128, KT, NH, HD + 1], F16, tag="v16")  # 16.3 KB/p
    E0 = keep.tile([128, KT, 2, 512], F16, tag="E0")        # 32 KB/p
    E1 = keep.tile([128, KT, 2, 512], F16, tag="E1")
    attn = keep.tile([128, HP, S], F16, tag="attn")         # 16 KB/p
    wq16 = keep.tile([128, 8, 2 * QF], F16, tag="wq16")     # 16 KB/p
    wv16 = keep.tile([128, 8, QF], F16, tag="wv16")         # 8 KB/p
    wo16 = keep.tile([128, 4, D], F16, tag="wo16")          # 8 KB/p
    bq_t = keep.tile([128, 8, 1], F32, tag="bq")
    E_bufs = (E0, E1)

    at_pool = ctx.enter_context(tc.tile_pool(name="at_pool", bufs=3))
    rec_pool = ctx.enter_context(tc.tile_pool(name="rec_pool", bufs=3))
    ostg_pool = ctx.enter_context(tc.tile_pool(name="ostg", bufs=2))
    ps_sc = ctx.enter_context(tc.tile_pool(name="ps_sc", bufs=2, space="PSUM"))
    ps_pv = ctx.enter_context(tc.tile_pool(name="ps_pv", bufs=2, space="PSUM"))
    ps_ms = ctx.enter_context(tc.tile_pool(name="ps_ms", bufs=2, space="PSUM"))

    nev = [0]  # psum-evacuation engine alternation

    def evac(o, in_, bias=None):
        nev[0] += 1
        if nev[0] % 2:
            if bias is not None:
                nc.scalar.activation(out=o, in_=in_, func=Identity,
                                     bias=bias, scale=1.0)
            else:
                nc.scalar.copy(out=o, in_=in_)
        elif bias is not None:
            nc.vector.tensor_scalar_add(out=o, in0=in_, scalar1=bias)
        else:
            nc.vector.tensor_copy(out=o, in_=in_)

    def emit_input_dmas():
        nc.sync.dma_start(out=bq_t, in_=bqk.rearrange("(f p) o -> p f o", p=128))
        nc.sync.dma_start(out=wq16, in_=wqk.rearrange("(dc p) f -> p dc f", p=128))
        nc.sync.dma_start(out=xt[:, :, 0:512],
                          in_=xT[:, 0:512].rearrange("(dc p) s -> p dc s", p=128))
        nc.sync.dma_start(out=wv16, in_=wv.rearrange("(dc p) f -> p dc f", p=128))
        for pc in range(1, 4):
            nc.sync.dma_start(
                out=xt[:, :, 512 * pc:512 * (pc + 1)],
                in_=xT[:, 512 * pc:512 * (pc + 1)].rearrange(
                    "(dc p) s -> p dc s", p=128))
        nc.sync.dma_start(out=wo16, in_=wo.rearrange("(fc p) d -> p fc d", p=128))

    def qk_use(F, ps):
        """qkT[:, F, ps*512:+512] = (x @ wqk[:, F-tile]) + bias (fp16)."""
        ms = ps_ms.tile([128, 512], F32, tag="ms", name=f"qk{F}_{ps}")
        for dc in range(8):
            nc.tensor.matmul(
                ms, wq16[:, dc, 128 * F:128 * (F + 1)],
                xt[:, dc, 512 * ps:512 * (ps + 1)],
                start=(dc == 0), stop=(dc == 7))
        evac(qkT[:, F, 512 * ps:512 * (ps + 1)], ms, bias=bq_t[:, F, :])

    def v_use(kt):
        """v16[:, kt, :, 0:64] = x[kt-tile] @ wv (fp16, natural layout)."""
        ms = ps_ms.tile([128, 512], F32, tag="ms", name=f"v{kt}")
        for dc in range(8):
            nc.tensor.matmul(ms, xt[:, dc, 128 * kt:128 * (kt + 1)],
                             wv16[:, dc, :], start=(dc == 0), stop=(dc == 7))
        evac(v16[:, kt, :, 0:HD], ms)

    def emit_scores(blk, kt):
        hp, qc = blk
        sc = ps_sc.tile([128, 2, 512], F32, tag="sc", name=f"sc{hp}_{qc}_{kt}")
        for j in range(2):
            p0 = 64 * j
            nc.tensor.matmul(
                sc[:, j, :],
                qkT[p0:p0 + 64, 4 + hp, 128 * kt:128 * (kt + 1)],
                qkT[p0:p0 + 64, hp, 512 * qc:512 * (qc + 1)],
                start=True, stop=True)
        return sc

    def emit_exp(E_sb, sc, kt):
        if kt < ACT_KTS:
            nc.scalar.activation(out=E_sb[:, kt, :, :], in_=sc, func=Exp,
                                 scale=0.125)
        else:
            nc.vector.tensor_scalar(
                out=E_sb[:, kt, :, :].bitcast(U16), in0=sc,
                scalar1=SCH_A, scalar2=SCH_B,
                op0=mybir.AluOpType.mult, op1=mybir.AluOpType.add)

    def pv_use(E_sb, hp, qc, u):
        """attn-T for q-tile u (128 wide) of block (hp, qc), both heads."""
        pv = ps_pv.tile([128, 512], F32, tag="pv", name=f"pv{hp}_{qc}_{u}")
        q0 = 128 * u
        for j in range(2):
            for kt in range(KT):
                nc.tensor.matmul(
                    pv[:, 128 * j:128 * j + HD + 1],
                    E_sb[:, kt, j, q0:q0 + 128],
                    v16[:, kt, 2 * hp + j, :],
                    start=(kt == 0), stop=(kt == KT - 1))
        pvr = pv.rearrange("p (c f) -> p c f", c=4)
        rec = rec_pool.tile([128, 2], F32, tag="rec")
        nc.vector.reciprocal(out=rec, in_=pvr[:, 0:2, HD:HD + 1])
        at = at_pool.tile([128, 2, 64], F16, tag="at")
        for j in range(2):
            nc.vector.tensor_scalar_mul(
                out=at[:, j, :], in0=pvr[:, j, 0:HD], scalar1=rec[:, j:j + 1])
        # XBAR transpose: [q 128, feat 128] -> attn[feat 128, q 128]
        qg = 512 * qc + q0
        nc.sync.dma_start_transpose(out=attn[:, hp, qg:qg + 128], in_=at)

    def op_use(qc, st, dh):
        """output projection: s-tile st of chunk qc, dout half dh (fp16)."""
        ms = ps_ms.tile([128, 512], F32, tag="ms", name=f"op{qc}_{st}_{dh}")
        s0 = 512 * qc + 128 * st
        for fc in range(4):
            nc.tensor.matmul(ms, attn[:, fc, s0:s0 + 128],
                             wo16[:, fc, 512 * dh:512 * (dh + 1)],
                             start=(fc == 0), stop=(fc == 3))
        if dh == 0:
            op_use.cur = ostg_pool.tile([128, D], F16, tag="ostg",
                                        name=f"os{qc}_{st}")
        evac(op_use.cur[:, 512 * dh:512 * (dh + 1)], ms)
        if dh == 1:
            nc.sync.dma_start(out=out_d[s0:s0 + 128, :], in_=op_use.cur)

    # ---- emission schedule --------------------------------------------------
    nc.vector.memset(v16[:, :, :, HD:HD + 1], 1.0)
    emit_input_dmas()

    # prologue: qk tiles needed by block (0,0), then block-(0,0) scores+exp
    # with all v tiles as fillers
    for F, ps in [(4, 0), (4, 1), (4, 2), (4, 3), (0, 0)]:
        qk_use(F, ps)
    for kt in range(KT):
        v_use(kt)
        sc = emit_scores((0, 0), kt)
        emit_exp(E_bufs[0], sc, kt)

    # q-feature tiles for block i+1's scores must precede block i's kt loop
    # (pre_fill); k-feature tiles covering kt-range 4*ps..4*ps+3 must precede
    # slot 4*ps of the emitting block (slot_fill).
    pre_fill = {
        0: [(0, 1)], 1: [(0, 2)], 2: [(0, 3)],
        3: [(1, 0), (5, 0)],
        4: [(1, 1)], 5: [(1, 2)], 6: [(1, 3)],
        7: [(2, 0), (6, 0)],
        8: [(2, 1)], 9: [(2, 2)], 10: [(2, 3)],
        11: [(3, 0), (7, 0)],
        12: [(3, 1)], 13: [(3, 2)], 14: [(3, 3)],
    }
    slot_fill = {}
    for bi, kf in ((3, 5), (7, 6), (11, 7)):
        slot_fill.update({(bi, 0): (kf, 1), (bi, 4): (kf, 2), (bi, 8): (kf, 3)})

    blocks = [(hp, qc) for hp in range(HP) for qc in range(QC)]
    for i, blk in enumerate(blocks):
        hp, qc = blk
        nxt = blocks[i + 1] if i + 1 < len(blocks) else None
        for f in pre_fill.get(i, []):
            qk_use(*f)
        ops = ([("op", i - 13, st, dh) for st in range(4) for dh in range(2)]
               if i >= 13 else [])
        for kt in range(KT):
            f = slot_fill.get((i, kt))
            if f is not None:
                qk_use(*f)
            elif kt % 2 == 0 and ops:
                o = ops.pop(0)
                op_use(o[1], o[2], o[3])
            if nxt is not None:
                sc = emit_scores(nxt, kt)
                emit_exp(E_bufs[(i + 1) % 2], sc, kt)
            if kt % 4 == 3:
                pv_use(E_bufs[i % 2], hp, qc, kt // 4)
        while ops:
            o = ops.pop(0)
            op_use(o[1], o[2], o[3])
    # epilogue: last output projection chunk
    for st in range(4):
        for dh in range(2):
            op_use(3, st, dh)


def _get_nc():
    if "nc" not in _CACHE:
        _CACHE["nc"] = _build()
    return _CACHE["nc"]


def _make_in_maps(x, w_qkv, b_qkv, w_out):
    in_maps = []
    for c in range(NCORES):
        b, half = divmod(c, 2)
        hs = half * QF
        wqk = np.concatenate([w_qkv[:, hs:hs + QF],
                              w_qkv[:, D + hs:D + hs + QF]], axis=1)
        bq = np.concatenate([b_qkv[hs:hs + QF], b_qkv[D + hs:D + hs + QF]])
        in_maps.append({
            "xT": np.ascontiguousarray(x[b].T).astype(np.float16),
            "wqk": wqk.astype(np.float16),
            "wv": np.ascontiguousarray(
                w_qkv[:, 2 * D + hs:2 * D + hs + QF]).astype(np.float16),
            "wo": np.ascontiguousarray(w_out[hs:hs + QF, :]).astype(np.float16),
            "bqk": bq[:, None].astype(np.float32),
        })
    return in_maps


def kernel(x, w_qkv, b_qkv, w_out, b_out):
    x = np.asarray(x, dtype=np.float32)
    w_qkv = np.asarray(w_qkv, dtype=np.float32)
    b_qkv = np.asarray(b_qkv, dtype=np.float32)
    w_out = np.asarray(w_out, dtype=np.float32)
    b_out = np.asarray(b_out, dtype=np.float32)

    nc = _get_nc()
    in_maps = _make_in_maps(x, w_qkv, b_qkv, w_out)
    res = run_bass_kernel_spmd(nc, in_maps, list(range(NCORES)))
    _CACHE["last_results"] = res

    const = b_out + b_qkv[2 * D:] @ w_out
    out = np.empty((B, S, D), dtype=np.float32)
    for b in range(B):
        out[b] = (res.results[2 * b]["out_partial"].astype(np.float32)
                  + res.results[2 * b + 1]["out_partial"].astype(np.float32)
                  + const)
    return out


# revision 8
# speedup vs baseline: 1.1396x; 1.0117x over previous
"""Trainium2 Bass kernel for multi-head self-attention (v3).

Problem: B=4, S=2048, D=1024, H=16 heads (HD=64), fp32 I/O.
Sharding: core c handles batch c//2, head-half c%2 (8 heads each); host
sums the two partial outputs per batch and adds constant bias terms.

v3 (cost-model driven, fp16 datapath for accuracy):
- all matmuls fp16 (fp8 weight/score noise does not average away
  relative to attention-output magnitude and blows the 2e-2 budget).
- exp split between ScalarE (exact exp) and VectorE (Schraudolph
  bit-trick: fp16 bits = round(s*184.66 + 15316), ~2% rms on 6/16 kt).
- PV transposed: out[q128, 65] with fused ones-column rowsum
  (65 cycles/instr instead of 512 for the natural layout);
  normalize via per-partition reciprocal+mul into attn^T;
  transpose back via XBAR DMA transpose (no PE, no PSUM).
- fp16 partial-output DMA; host combines in fp32.
"""

import contextlib
import numpy as np

import concourse.bacc as bacc
import concourse.tile as tile
from concourse import mybir
from concourse.bass_utils import run_bass_kernel_spmd

B, S, D, H, HD = 4, 2048, 1024, 16, 64
NCORES = 8
NH = 8              # heads per core
QF = 512            # q features per core
KT = 16             # 128-wide key position tiles
QC = 4              # 512-wide q chunks
HP = 4              # head pairs
F32 = mybir.dt.float32
F16 = mybir.dt.float16
U16 = mybir.dt.uint16
Exp = mybir.ActivationFunctionType.Exp
Identity = mybir.ActivationFunctionType.Identity

LOG2E = 1.4426950408889634
SCH_A = 0.125 * 1024 * LOG2E    # schraudolph fp16-bits slope
SCH_B = 15360.0 - 44.0          # bias 15, centered correction
ACT_KTS = 10                    # kt < ACT_KTS -> exp on ScalarE, else DVE

_CACHE = {}


def _build():
    nc = bacc.Bacc("TRN2", target_bir_lowering=False, debug=False)

    xT = nc.dram_tensor("xT", [D, S], F16, kind="ExternalInput").ap()
    wqk = nc.dram_tensor("wqk", [D, 2 * QF], F16, kind="ExternalInput").ap()
    wv = nc.dram_tensor("wv", [D, QF], F16, kind="ExternalInput").ap()
    wo = nc.dram_tensor("wo", [QF, D], F16, kind="ExternalInput").ap()
    bqk = nc.dram_tensor("bqk", [2 * QF, 1], F32, kind="ExternalInput").ap()
    out_d = nc.dram_tensor("out_partial", [S, D], F16, kind="ExternalOutput").ap()

    with tile.TileContext(nc) as tc:
        with contextlib.ExitStack() as ctx:
            with nc.allow_low_precision(reason="fp16 intermediates"):
                _emit(nc, tc, ctx, xT, wqk, wv, wo, bqk, out_d)
    nc.compile()
    return nc


def _emit(nc, tc, ctx, xT, wqk, wv, wo, bqk, out_d):
    keep = ctx.enter_context(tc.tile_pool(name="keep", bufs=1))
    xt = keep.tile([128, 8, S], F16, tag="xt")              # 32 KB/p
    qkT = keep.tile([128, 8, S], F16, tag="qkT")            # 32 KB/p
    v16 = keep.tile([128, KT, NH, HD + 1], F16, tag="v16")  # 16.3 KB/p
    E0 = keep.tile([128, KT, 2, 512], F16, tag="E0")        # 32 KB/p
    E1 = keep.tile([128, KT, 2, 512], F16, tag="E1")
    attn = keep.tile([128, HP, S], F16, tag="attn")         # 16 KB/p
    wq16 = keep.tile([128, 8, 2 * QF], F16, tag="wq16")     # 16 KB/p
    wv16 = keep.tile([128, 8, QF], F16, tag="wv16")         # 8 KB/p
    wo16 = keep.tile([128, 4, D], F16, tag="wo16")          # 8 KB/p
    bq_t = keep.tile([128, 8, 1], F32, tag="bq")
    E_bufs = (E0, E1)

    at_pool = ctx.enter_context(tc.tile_pool(name="at_pool", bufs=3))
    rec_pool = ctx.enter_context(tc.tile_pool(name="rec_pool", bufs=3))
    ostg_pool = ctx.enter_context(tc.tile_pool(name="ostg", bufs=2))
    ps_sc = ctx.enter_context(tc.tile_pool(name="ps_sc", bufs=2, space="PSUM"))
    ps_pv = ctx.enter_context(tc.tile_pool(name="ps_pv", bufs=2, space="PSUM"))
    ps_ms = ctx.enter_context(tc.tile_pool(name="ps_ms", bufs=2, space="PSUM"))

    nev = [0]  # psum-evacuation engine alternation

    def evac(o, in_, bias=None):
        nev[0] += 1
        if nev[0] % 2:
            if bias is not None:
                nc.scalar.activation(out=o, in_=in_, func=Identity,
                                     bias=bias, scale=1.0)
            else:
                nc.scalar.copy(out=o, in_=in_)
        elif bias is not None:
            nc.vector.tensor_scalar_add(out=o, in0=in_, scalar1=bias)
        else:
            nc.vector.tensor_copy(out=o, in_=in_)

    def emit_input_dmas():
        nc.sync.dma_start(out=bq_t, in_=bqk.rearrange("(f p) o -> p f o", p=128))
        nc.sync.dma_start(out=wq16, in_=wqk.rearrange("(dc p) f -> p dc f", p=128))
        nc.sync.dma_start(out=xt[:, :, 0:512],
                          in_=xT[:, 0:512].rearrange("(dc p) s -> p dc s", p=128))
        nc.sync.dma_start(out=wv16, in_=wv.rearrange("(dc p) f -> p dc f", p=128))
        for pc in range(1, 4):
            nc.sync.dma_start(
                out=xt[:, :, 512 * pc:512 * (pc + 1)],
                in_=xT[:, 512 * pc:512 * (pc + 1)].rearrange(
                    "(dc p) s -> p dc s", p=128))
        nc.sync.dma_start(out=wo16, in_=wo.rearrange("(fc p) d -> p fc d", p=128))

    def qk_use(F, ps):
        """qkT[:, F, ps*512:+512] = (x @ wqk[:, F-tile]) + bias (fp16)."""
        ms = ps_ms.tile([128, 512], F32, tag="ms", name=f"qk{F}_{ps}")
        for dc in range(8):
            nc.tensor.matmul(
                ms, wq16[:, dc, 128 * F:128 * (F + 1)],
                xt[:, dc, 512 * ps:512 * (ps + 1)],
                start=(dc == 0), stop=(dc == 7))
        evac(qkT[:, F, 512 * ps:512 * (ps + 1)], ms, bias=bq_t[:, F, :])

    def v_use(kt):
        """v16[:, kt, :, 0:64] = x[kt-tile] @ wv (fp16, natural layout)."""
        ms = ps_ms.tile([128, 512], F32, tag="ms", name=f"v{kt}")
        for dc in range(8):
            nc.tensor.matmul(ms, xt[:, dc, 128 * kt:128 * (kt + 1)],
                             wv16[:, dc, :], start=(dc == 0), stop=(dc == 7))
        evac(v16[:, kt, :, 0:HD], ms)

    def emit_scores(blk, kt):
        hp, qc = blk
        sc = ps_sc.tile([128, 2, 512], F32, tag="sc", name=f"sc{hp}_{qc}_{kt}")
        for j in range(2):
            p0 = 64 * j
            nc.tensor.matmul(
                sc[:, j, :],
                qkT[p0:p0 + 64, 4 + hp, 128 * kt:128 * (kt + 1)],
                qkT[p0:p0 + 64, hp, 512 * qc:512 * (qc + 1)],
                start=True, stop=True)
        return sc

    def emit_exp(E_sb, sc, kt):
        # split each tile across both engines so neither paces the pipeline:
        # ScalarE exact exp on head j0, VectorE Schraudolph bits on head j1
        nc.scalar.activation(out=E_sb[:, kt, 0, :], in_=sc[:, 0, :], func=Exp,
                             scale=0.125)
        nc.vector.tensor_scalar(
            out=E_sb[:, kt, 1, :].bitcast(U16), in0=sc[:, 1, :],
            scalar1=SCH_A, scalar2=SCH_B,
            op0=mybir.AluOpType.mult, op1=mybir.AluOpType.add)

    def pv_use(E_sb, hp, qc, u):
        """attn-T for q-tile u (128 wide) of block (hp, qc), both heads."""
        pv = ps_pv.tile([128, 512], F32, tag="pv", name=f"pv{hp}_{qc}_{u}")
        q0 = 128 * u
        for j in range(2):
            for kt in range(KT):
                nc.tensor.matmul(
                    pv[:, 128 * j:128 * j + HD + 1],
                    E_sb[:, kt, j, q0:q0 + 128],
                    v16[:, kt, 2 * hp + j, :],
                    start=(kt == 0), stop=(kt == KT - 1))
        pvr = pv.rearrange("p (c f) -> p c f", c=4)
        rec = rec_pool.tile([128, 2], F32, tag="rec")
        nc.vector.reciprocal(out=rec, in_=pvr[:, 0:2, HD:HD + 1])
        at = at_pool.tile([128, 2, 64], F16, tag="at")
        for j in range(2):
            if j == 0:
                nc.scalar.activation(
                    out=at[:, j, :], in_=pvr[:, j, 0:HD],
                    func=mybir.ActivationFunctionType.Copy,
                    bias=0.0, scale=rec[:, j:j + 1])
            else:
                nc.vector.tensor_scalar_mul(
                    out=at[:, j, :], in0=pvr[:, j, 0:HD],
                    scalar1=rec[:, j:j + 1])
        # XBAR transpose: [q 128, feat 128] -> attn[feat 128, q 128]
        qg = 512 * qc + q0
        nc.sync.dma_start_transpose(out=attn[:, hp, qg:qg + 128], in_=at)

    def op_use(qc, st, dh):
        """output projection: s-tile st of chunk qc, dout half dh (fp16)."""
        ms = ps_ms.tile([128, 512], F32, tag="ms", name=f"op{qc}_{st}_{dh}")
        s0 = 512 * qc + 128 * st
        for fc in range(4):
            nc.tensor.matmul(ms, attn[:, fc, s0:s0 + 128],
                             wo16[:, fc, 512 * dh:512 * (dh + 1)],
                             start=(fc == 0), stop=(fc == 3))
        if dh == 0:
            op_use.cur = ostg_pool.tile([128, D], F16, tag="ostg",
                                        name=f"os{qc}_{st}")
        evac(op_use.cur[:, 512 * dh:512 * (dh + 1)], ms)
        if dh == 1:
            nc.sync.dma_start(out=out_d[s0:s0 + 128, :], in_=op_use.cur)

    # ---- emission schedule --------------------------------------------------
    nc.vector.memset(v16[:, :, :, HD:HD + 1], 1.0)
    emit_input_dmas()

    # prologue: qk tiles needed by block (0,0), then block-(0,0) scores+exp
    # with all v tiles as fillers
    for F, ps in [(4, 0), (4, 1), (4, 2), (4, 3), (0, 0)]:
        qk_use(F, ps)
    for kt in range(KT):
        v_use(kt)
        sc = emit_scores((0, 0), kt)
        emit_exp(E_bufs[0], sc, kt)

    # q-feature tiles for block i+1's scores must precede block i's kt loop
    # (pre_fill); k-feature tiles covering kt-range 4*ps..4*ps+3 must precede
    # slot 4*ps of the emitting block (slot_fill).
    pre_fill = {
        0: [(0, 1)], 1: [(0, 2)], 2: [(0, 3)],
        3: [(1, 0), (5, 0)],
        4: [(1, 1)], 5: [(1, 2)], 6: [(1, 3)],
        7: [(2, 0), (6, 0)],
        8: [(2, 1)], 9: [(2, 2)], 10: [(2, 3)],
        11: [(3, 0), (7, 0)],
        12: [(3, 1)], 13: [(3, 2)], 14: [(3, 3)],
    }
    slot_fill = {}
    for bi, kf in ((3, 5), (7, 6), (11, 7)):
        slot_fill.update({(bi, 0): (kf, 1), (bi, 4): (kf, 2), (bi, 8): (kf, 3)})

    blocks = [(hp, qc) for hp in range(HP) for qc in range(QC)]
    for i, blk in enumerate(blocks):
        hp, qc = blk
        nxt = blocks[i + 1] if i + 1 < len(blocks) else None
        for f in pre_fill.get(i, []):
            qk_use(*f)
        ops = ([("op", i - 13, st, dh) for st in range(4) for dh in range(2)]
               if i >= 13 else [])
        for kt in range(KT):
            f = slot_fill.get((i, kt))
            if f is not None:
                qk_use(*f)
            elif kt % 2 == 0 and ops:
                o = ops.pop(0)
                op_use(o[1], o[2], o[3])
            if nxt is not None:
                sc = emit_scores(nxt, kt)
                emit_exp(E_bufs[(i + 1) % 2], sc, kt)
            if kt % 4 == 3:
                pv_use(E_bufs[i % 2], hp, qc, kt // 4)
        while ops:
            o = ops.pop(0)
            op_use(o[1], o[2], o[3])
    # epilogue: last output projection chunk
    for st in range(4):
        for dh in range(2):
            op_use(3, st, dh)


def _get_nc():
    if "nc" not in _CACHE:
        _CACHE["nc"] = _build()
    return _CACHE["nc"]


def _make_in_maps(x, w_qkv, b_qkv, w_out):
    in_maps = []
    for c in range(NCORES):
        b, half = divmod(c, 2)
        hs = half * QF
        wqk = np.concatenate([w_qkv[:, hs:hs + QF],
                              w_qkv[:, D + hs:D + hs + QF]], axis=1)
        bq = np.concatenate([b_qkv[hs:hs + QF], b_qkv[D + hs:D + hs + QF]])
        in_maps.append({
            "xT": np.ascontiguousarray(x[b].T).astype(np.float16),
            "wqk": wqk.astype(np.float16),
            "wv": np.ascontiguousarray(
                w_qkv[:, 2 * D + hs:2 * D + hs + QF]).astype(np.float16),
            "wo": np.ascontiguousarray(w_out[hs:hs + QF, :]).astype(np.float16),
            "bqk": bq[:, None].astype(np.float32),
        })
    return in_maps


def kernel(x, w_qkv, b_qkv, w_out, b_out):
    x = np.asarray(x, dtype=np.float32)
    w_qkv = np.asarray(w_qkv, dtype=np.float32)
    b_qkv = np.asarray(b_qkv, dtype=np.float32)
    w_out = np.asarray(w_out, dtype=np.float32)
    b_out = np.asarray(b_out, dtype=np.float32)

    nc = _get_nc()
    in_maps = _make_in_maps(x, w_qkv, b_qkv, w_out)
    res = run_bass_kernel_spmd(nc, in_maps, list(range(NCORES)))
    _CACHE["last_results"] = res

    const = b_out + b_qkv[2 * D:] @ w_out
    out = np.empty((B, S, D), dtype=np.float32)
    for b in range(B):
        out[b] = (res.results[2 * b]["out_partial"].astype(np.float32)
                  + res.results[2 * b + 1]["out_partial"].astype(np.float32)
                  + const)
    return out


# revision 9
# speedup vs baseline: 1.2595x; 1.1052x over previous
"""Trainium2 Bass kernel for multi-head self-attention (v3).

Problem: B=4, S=2048, D=1024, H=16 heads (HD=64), fp32 I/O.
Sharding: core c handles batch c//2, head-half c%2 (8 heads each); host
sums the two partial outputs per batch and adds constant bias terms.

v3 (cost-model driven, fp16 datapath for accuracy):
- all matmuls fp16 (fp8 weight/score noise does not average away
  relative to attention-output magnitude and blows the 2e-2 budget).
- exp split between ScalarE (exact exp) and VectorE (Schraudolph
  bit-trick: fp16 bits = round(s*184.66 + 15316), ~2% rms on 6/16 kt).
- PV transposed: out[q128, 65] with fused ones-column rowsum
  (65 cycles/instr instead of 512 for the natural layout);
  normalize via per-partition reciprocal+mul into attn^T;
  transpose back via XBAR DMA transpose (no PE, no PSUM).
- fp16 partial-output DMA; host combines in fp32.
"""

import contextlib
import numpy as np

import concourse.bacc as bacc
import concourse.tile as tile
from concourse import mybir
from concourse.bass_utils import run_bass_kernel_spmd

B, S, D, H, HD = 4, 2048, 1024, 16, 64
NCORES = 8
NH = 8              # heads per core
QF = 512            # q features per core
KT = 16             # 128-wide key position tiles
QC = 4              # 512-wide q chunks
HP = 4              # head pairs
F32 = mybir.dt.float32
F16 = mybir.dt.float16
U16 = mybir.dt.uint16
Exp = mybir.ActivationFunctionType.Exp
Identity = mybir.ActivationFunctionType.Identity

LOG2E = 1.4426950408889634
SCH_A = 0.125 * 1024 * LOG2E    # schraudolph fp16-bits slope
SCH_B = 15360.0 - 44.0          # bias 15, centered correction
ACT_KTS = 10                    # kt < ACT_KTS -> exp on ScalarE, else DVE

_CACHE = {}


def _build():
    nc = bacc.Bacc("TRN2", target_bir_lowering=False, debug=False)

    xT = nc.dram_tensor("xT", [D, S], F16, kind="ExternalInput").ap()
    wqk = nc.dram_tensor("wqk", [D, 2 * QF], F16, kind="ExternalInput").ap()
    wv = nc.dram_tensor("wv", [D, QF], F16, kind="ExternalInput").ap()
    wo = nc.dram_tensor("wo", [QF, D], F16, kind="ExternalInput").ap()
    bqk = nc.dram_tensor("bqk", [2 * QF, 1], F32, kind="ExternalInput").ap()
    out_d = nc.dram_tensor("out_partial", [S, D], F16, kind="ExternalOutput").ap()

    with tile.TileContext(nc) as tc:
        with contextlib.ExitStack() as ctx:
            with nc.allow_low_precision(reason="fp16 intermediates"):
                _emit(nc, tc, ctx, xT, wqk, wv, wo, bqk, out_d)
    nc.compile()
    return nc


def _emit(nc, tc, ctx, xT, wqk, wv, wo, bqk, out_d):
    keep = ctx.enter_context(tc.tile_pool(name="keep", bufs=1))
    xt = keep.tile([128, 8, S], F16, tag="xt")              # 32 KB/p
    qkT = keep.tile([128, 8, S], F16, tag="qkT")            # 32 KB/p
    v16 = keep.tile([128, KT, NH, HD + 1], F16, tag="v16")  # 16.3 KB/p
    E0 = keep.tile([128, KT, 2, 512], F16, tag="E0")        # 32 KB/p
    E1 = keep.tile([128, KT, 2, 512], F16, tag="E1")
    attn = keep.tile([128, HP, S], F16, tag="attn")         # 16 KB/p
    wq16 = keep.tile([128, 8, 2 * QF], F16, tag="wq16")     # 16 KB/p
    wv16 = keep.tile([128, 8, QF], F16, tag="wv16")         # 8 KB/p
    wo16 = keep.tile([128, 4, D], F16, tag="wo16")          # 8 KB/p
    bq_t = keep.tile([128, 8, 1], F32, tag="bq")
    E_bufs = (E0, E1)

    at_pool = ctx.enter_context(tc.tile_pool(name="at_pool", bufs=3))
    rec_pool = ctx.enter_context(tc.tile_pool(name="rec_pool", bufs=3))
    ostg_pool = ctx.enter_context(tc.tile_pool(name="ostg", bufs=2))
    ps_sc = ctx.enter_context(tc.tile_pool(name="ps_sc", bufs=4, space="PSUM"))
    ps_pv = ctx.enter_context(tc.tile_pool(name="ps_pv", bufs=2, space="PSUM"))
    ps_ms = ctx.enter_context(tc.tile_pool(name="ps_ms", bufs=2, space="PSUM"))

    nev = [0]  # psum-evacuation engine alternation

    def evac(o, in_, bias=None):
        nev[0] += 1
        if nev[0] % 2:
            if bias is not None:
                nc.scalar.activation(out=o, in_=in_, func=Identity,
                                     bias=bias, scale=1.0)
            else:
                nc.scalar.copy(out=o, in_=in_)
        elif bias is not None:
            nc.vector.tensor_scalar_add(out=o, in0=in_, scalar1=bias)
        else:
            nc.vector.tensor_copy(out=o, in_=in_)

    def emit_input_dmas():
        nc.sync.dma_start(out=bq_t, in_=bqk.rearrange("(f p) o -> p f o", p=128))
        nc.sync.dma_start(out=wq16, in_=wqk.rearrange("(dc p) f -> p dc f", p=128))
        nc.sync.dma_start(out=xt[:, :, 0:512],
                          in_=xT[:, 0:512].rearrange("(dc p) s -> p dc s", p=128))
        nc.sync.dma_start(out=wv16, in_=wv.rearrange("(dc p) f -> p dc f", p=128))
        for pc in range(1, 4):
            nc.sync.dma_start(
                out=xt[:, :, 512 * pc:512 * (pc + 1)],
                in_=xT[:, 512 * pc:512 * (pc + 1)].rearrange(
                    "(dc p) s -> p dc s", p=128))
        nc.sync.dma_start(out=wo16, in_=wo.rearrange("(fc p) d -> p fc d", p=128))

    def qk_use(F, ps):
        """qkT[:, F, ps*512:+512] = (x @ wqk[:, F-tile]) + bias (fp16)."""
        ms = ps_ms.tile([128, 512], F32, tag="ms", name=f"qk{F}_{ps}")
        for dc in range(8):
            nc.tensor.matmul(
                ms, wq16[:, dc, 128 * F:128 * (F + 1)],
                xt[:, dc, 512 * ps:512 * (ps + 1)],
                start=(dc == 0), stop=(dc == 7))
        evac(qkT[:, F, 512 * ps:512 * (ps + 1)], ms, bias=bq_t[:, F, :])

    def v_use(kt):
        """v16[:, kt, :, 0:64] = x[kt-tile] @ wv (fp16, natural layout)."""
        ms = ps_ms.tile([128, 512], F32, tag="ms", name=f"v{kt}")
        for dc in range(8):
            nc.tensor.matmul(ms, xt[:, dc, 128 * kt:128 * (kt + 1)],
                             wv16[:, dc, :], start=(dc == 0), stop=(dc == 7))
        evac(v16[:, kt, :, 0:HD], ms)

    def emit_scores(blk, kt):
        hp, qc = blk
        sc = [ps_sc.tile([128, 512], F32, tag="sc", name=f"sc{hp}_{qc}_{kt}_{j}")
              for j in range(2)]
        for j in range(2):
            p0 = 64 * j
            nc.tensor.matmul(
                sc[j],
                qkT[p0:p0 + 64, 4 + hp, 128 * kt:128 * (kt + 1)],
                qkT[p0:p0 + 64, hp, 512 * qc:512 * (qc + 1)],
                start=True, stop=True)
        return sc

    def emit_exp(E_sb, sc, kt):
        # split each tile across both engines so neither paces the pipeline:
        # ScalarE exact exp on head j0, VectorE Schraudolph bits on head j1
        nc.scalar.activation(out=E_sb[:, kt, 0, :], in_=sc[0], func=Exp,
                             scale=0.125)
        nc.vector.tensor_scalar(
            out=E_sb[:, kt, 1, :].bitcast(U16), in0=sc[1],
            scalar1=SCH_A, scalar2=SCH_B,
            op0=mybir.AluOpType.mult, op1=mybir.AluOpType.add)

    def pv_use(E_sb, hp, qc, u):
        """attn-T for q-tile u (128 wide) of block (hp, qc), both heads."""
        pv = ps_pv.tile([128, 512], F32, tag="pv", name=f"pv{hp}_{qc}_{u}")
        q0 = 128 * u
        for j in range(2):
            for kt in range(KT):
                nc.tensor.matmul(
                    pv[:, 128 * j:128 * j + HD + 1],
                    E_sb[:, kt, j, q0:q0 + 128],
                    v16[:, kt, 2 * hp + j, :],
                    start=(kt == 0), stop=(kt == KT - 1))
        pvr = pv.rearrange("p (c f) -> p c f", c=4)
        rec = rec_pool.tile([128, 2], F32, tag="rec")
        nc.vector.reciprocal(out=rec, in_=pvr[:, 0:2, HD:HD + 1])
        at = at_pool.tile([128, 2, 64], F16, tag="at")
        for j in range(2):
            if j == 0:
                nc.scalar.activation(
                    out=at[:, j, :], in_=pvr[:, j, 0:HD],
                    func=mybir.ActivationFunctionType.Copy,
                    bias=0.0, scale=rec[:, j:j + 1])
            else:
                nc.vector.tensor_scalar_mul(
                    out=at[:, j, :], in0=pvr[:, j, 0:HD],
                    scalar1=rec[:, j:j + 1])
        # XBAR transpose: [q 128, feat 128] -> attn[feat 128, q 128]
        qg = 512 * qc + q0
        nc.sync.dma_start_transpose(out=attn[:, hp, qg:qg + 128], in_=at)

    def op_use(qc, st, dh):
        """output projection: s-tile st of chunk qc, dout half dh (fp16)."""
        ms = ps_ms.tile([128, 512], F32, tag="ms", name=f"op{qc}_{st}_{dh}")
        s0 = 512 * qc + 128 * st
        for fc in range(4):
            nc.tensor.matmul(ms, attn[:, fc, s0:s0 + 128],
                             wo16[:, fc, 512 * dh:512 * (dh + 1)],
                             start=(fc == 0), stop=(fc == 3))
        if dh == 0:
            op_use.cur = ostg_pool.tile([128, D], F16, tag="ostg",
                                        name=f"os{qc}_{st}")
        evac(op_use.cur[:, 512 * dh:512 * (dh + 1)], ms)
        if dh == 1:
            nc.sync.dma_start(out=out_d[s0:s0 + 128, :], in_=op_use.cur)

    # ---- emission schedule --------------------------------------------------
    nc.vector.memset(v16[:, :, :, HD:HD + 1], 1.0)
    emit_input_dmas()

    # prologue: qk tiles needed by block (0,0), then block-(0,0) scores+exp
    # with all v tiles as fillers
    for F, ps in [(4, 0), (4, 1), (4, 2), (4, 3), (0, 0)]:
        qk_use(F, ps)
    for kt in range(KT):
        v_use(kt)
        sc = emit_scores((0, 0), kt)
        emit_exp(E_bufs[0], sc, kt)

    # q-feature tiles for block i+1's scores must precede block i's kt loop
    # (pre_fill); k-feature tiles covering kt-range 4*ps..4*ps+3 must precede
    # slot 4*ps of the emitting block (slot_fill).
    pre_fill = {
        0: [(0, 1)], 1: [(0, 2)], 2: [(0, 3)],
        3: [(1, 0), (5, 0)],
        4: [(1, 1)], 5: [(1, 2)], 6: [(1, 3)],
        7: [(2, 0), (6, 0)],
        8: [(2, 1)], 9: [(2, 2)], 10: [(2, 3)],
        11: [(3, 0), (7, 0)],
        12: [(3, 1)], 13: [(3, 2)], 14: [(3, 3)],
    }
    slot_fill = {}
    for bi, kf in ((3, 5), (7, 6), (11, 7)):
        slot_fill.update({(bi, 0): (kf, 1), (bi, 4): (kf, 2), (bi, 8): (kf, 3)})

    blocks = [(hp, qc) for hp in range(HP) for qc in range(QC)]
    for i, blk in enumerate(blocks):
        hp, qc = blk
        nxt = blocks[i + 1] if i + 1 < len(blocks) else None
        for f in pre_fill.get(i, []):
            qk_use(*f)
        ops = ([("op", i - 13, st, dh) for st in range(4) for dh in range(2)]
               if i >= 13 else [])
        for kt in range(KT):
            f = slot_fill.get((i, kt))
            if f is not None:
                qk_use(*f)
            elif kt % 2 == 0 and ops:
                o = ops.pop(0)
                op_use(o[1], o[2], o[3])
            if nxt is not None:
                sc = emit_scores(nxt, kt)
                emit_exp(E_bufs[(i + 1) % 2], sc, kt)
            if kt % 4 == 3:
                pv_use(E_bufs[i % 2], hp, qc, kt // 4)
        while ops:
            o = ops.pop(0)
            op_use(o[1], o[2], o[3])
    # epilogue: last output projection chunk
    for st in range(4):
        for dh in range(2):
            op_use(3, st, dh)


def _get_nc():
    if "nc" not in _CACHE:
        _CACHE["nc"] = _build()
    return _CACHE["nc"]


def _make_in_maps(x, w_qkv, b_qkv, w_out):
    in_maps = []
    for c in range(NCORES):
        b, half = divmod(c, 2)
        hs = half * QF
        wqk = np.concatenate([w_qkv[:, hs:hs + QF],
                              w_qkv[:, D + hs:D + hs + QF]], axis=1)
        bq = np.concatenate([b_qkv[hs:hs + QF], b_qkv[D + hs:D + hs + QF]])
        in_maps.append({
            "xT": np.ascontiguousarray(x[b].T).astype(np.float16),
            "wqk": wqk.astype(np.float16),
            "wv": np.ascontiguousarray(
                w_qkv[:, 2 * D + hs:2 * D + hs + QF]).astype(np.float16),
            "wo": np.ascontiguousarray(w_out[hs:hs + QF, :]).astype(np.float16),
            "bqk": bq[:, None].astype(np.float32),
        })
    return in_maps


def kernel(x, w_qkv, b_qkv, w_out, b_out):
    x = np.asarray(x, dtype=np.float32)
    w_qkv = np.asarray(w_qkv, dtype=np.float32)
    b_qkv = np.asarray(b_qkv, dtype=np.float32)
    w_out = np.asarray(w_out, dtype=np.float32)
    b_out = np.asarray(b_out, dtype=np.float32)

    nc = _get_nc()
    in_maps = _make_in_maps(x, w_qkv, b_qkv, w_out)
    res = run_bass_kernel_spmd(nc, in_maps, list(range(NCORES)))
    _CACHE["last_results"] = res

    const = b_out + b_qkv[2 * D:] @ w_out
    out = np.empty((B, S, D), dtype=np.float32)
    for b in range(B):
        out[b] = (res.results[2 * b]["out_partial"].astype(np.float32)
                  + res.results[2 * b + 1]["out_partial"].astype(np.float32)
                  + const)
    return out
